# revision 1
# baseline (speedup 1.0000x reference)
"""Two-layer GAT on 8 Trainium2 NeuronCores (Bass/Tile, SPMD).

Sharding: dst nodes split into 784 tiles of 128; core c owns the 98
contiguous tiles = nodes [c*12544, (c+1)*12544).  Edges (incl.
self-loops) are grouped by dst tile, padded to a uniform 19 chunks of
128 edges per tile, so the device program is input-independent.

Per chunk: indirect-DMA gather of source rows, is_equal one-hot
(edge -> local dst), PE transpose of the one-hot to expand the tile's
contiguous dst scores to edges, then one PE matmul segment-reduces the
ex-scaled features plus the softmax denominators into PSUM.  Segment
max is algebraically dropped (scores are O(1), exp cannot overflow;
softmax is shift-invariant so results match to rounding).  Between
layers the per-shard [h2 | s_src2 | s_dst2] tables are AllGathered.
"""
import os
import sys

sys.path.insert(0, "/opt/trn_rl_repo")

import numpy as np

N = 100000
IN_DIM = 128
HID = 32
HEADS = 4
OUT_DIM = 32
NEG_SLOPE = 0.2

NC = 8
P = 128
NPAD = 100352          # 784 tiles of 128
SHARD = NPAD // NC     # 12544
NS = SHARD // P        # 98 dst tiles per core
CS = 19                # chunks of 128 edges per dst tile
TC = NS * CS           # 1862 chunks per core
NT = NPAD // P         # 784
W1C = 136              # h1(128) | ssrc1(4) | sdst1(4)
W2C = 36               # h2(32) | ssrc2(1) | sdst2(1) | pad(2)

_RUNNER = None


def _ap(t, ap_dims, extra_offset=0):
    import concourse.bass as bass
    base = t[:]
    return bass.AP(base.tensor, base.offset + extra_offset, ap_dims)


def _build_program(ns_run=NS, p0_groups=None):
    from concourse import bass, mybir, bacc
    import concourse.tile as tile
    from concourse.masks import make_identity

    f32 = mybir.dt.float32
    bf16 = mybir.dt.bfloat16
    i32 = mybir.dt.int32
    AF = mybir.ActivationFunctionType
    ALU = mybir.AluOpType

    nc = bacc.Bacc("TRN2", target_bir_lowering=False, debug=False, num_devices=NC)

    xT = nc.dram_tensor("xT", [P, NPAD], bf16, kind="ExternalInput")
    W1cat = nc.dram_tensor("W1cat", [P, W1C], bf16, kind="ExternalInput")
    W2cat = nc.dram_tensor("W2cat", [P, W2C], f32, kind="ExternalInput")
    esrc = nc.dram_tensor("esrc", [P, TC], i32, kind="ExternalInput")
    edloc = nc.dram_tensor("edloc", [P, TC], f32, kind="ExternalInput")
    dtids = nc.dram_tensor("dtids", [P, NS], i32, kind="ExternalInput")
    out2 = nc.dram_tensor("out2", [SHARD, OUT_DIM], f32, kind="ExternalOutput")
    dbg = os.environ.get("GAT_DEBUG") == "1"
    if dbg:
        h1dbg = nc.dram_tensor("h1dbg", [1024, W1C], bf16, kind="ExternalOutput")
        h2dbg = nc.dram_tensor("h2dbg", [SHARD, W2C], bf16, kind="ExternalOutput")
        sd_d = nc.dram_tensor("sd_d", [P, CS * 4], f32, kind="ExternalOutput")
        s_d = nc.dram_tensor("s_d", [P, CS * 4], f32, kind="ExternalOutput")
        ex_d = nc.dram_tensor("ex_d", [P, CS * 4], f32, kind="ExternalOutput")
        m_d = nc.dram_tensor("m_d", [P, CS * P], f32, kind="ExternalOutput")
        g_d = nc.dram_tensor("g_d", [P, CS * W1C], f32, kind="ExternalOutput")
        agg_d = nc.dram_tensor("agg_d", [P, 132], f32, kind="ExternalOutput")
        sdt_d = nc.dram_tensor("sdt_d", [P, W1C], f32, kind="ExternalOutput")
    h1ext = nc.dram_tensor("h1ext", [NPAD, W1C], bf16)

    with tile.TileContext(nc) as tc:
        with (
            tc.tile_pool(name="consts", bufs=1) as consts,
            tc.tile_pool(name="sb", bufs=4) as sb,
            tc.tile_pool(name="gx", bufs=3) as gx,
            tc.tile_pool(name="ps", bufs=2, space="PSUM") as ps,
            tc.tile_pool(name="pst", bufs=2, space="PSUM") as pst,
            tc.tile_pool(name="psagg", bufs=2, space="PSUM") as psagg,
            tc.tile_pool(name="dram", bufs=1, space="DRAM") as dram,
        ):
            ident = consts.tile([P, P], f32)
            make_identity(nc, ident[:])
            identb = consts.tile([P, P], bf16)
            nc.vector.tensor_copy(out=identb[:], in_=ident[:])
            iota_i = consts.tile([P, P], i32)
            nc.gpsimd.iota(iota_i[:], pattern=[[1, P]], base=0, channel_multiplier=0)
            iota_f = consts.tile([P, P], f32)
            nc.vector.tensor_copy(out=iota_f[:], in_=iota_i[:])
            w1_t = consts.tile([P, W1C], bf16)
            nc.sync.dma_start(out=w1_t[:], in_=W1cat[:])
            w2_t = consts.tile([P, W2C], f32)
            nc.sync.dma_start(out=w2_t[:], in_=W2cat[:])
            dt_t = consts.tile([P, NS], i32)
            nc.sync.dma_start(out=dt_t[:], in_=dtids[:])

            # ---------- phase 0: h1ext = [x@W1 | x@W1s | x@W1d], all nodes
            GRP = 8
            _ng = NT // GRP if p0_groups is None else p0_groups
            for g in range(_ng):
                xg = gx.tile([P, P * GRP], bf16, tag="xg")
                nc.sync.dma_start(out=xg[:], in_=xT[:, g * P * GRP:(g + 1) * P * GRP])
                for t in range(GRP):
                    p0 = ps.tile([P, W1C], f32, tag="p0")
                    nc.tensor.matmul(out=p0[:], lhsT=xg[:, t * P:(t + 1) * P],
                                     rhs=w1_t[:], start=True, stop=True)
                    s0 = sb.tile([P, W1C], bf16, tag="s0")
                    nc.scalar.copy(out=s0[:], in_=p0[:])
                    nc.sync.dma_start(
                        out=h1ext[(g * GRP + t) * P:(g * GRP + t + 1) * P, :],
                        in_=s0[:])

            h2sh = dram.tile([SHARD, W2C], bf16)
            h2full = dram.tile([NPAD, W2C], bf16)

            # ---------- layer 1 edge pass over own dst tiles
            for s in range(ns_run):
                c0 = s * CS
                # this slot's node rows (for sdst1, cols 132:136)
                sdt = sb.tile([P, W1C], bf16, tag="sdt")
                nc.gpsimd.indirect_dma_start(
                    out=sdt[:], out_offset=None, in_=h1ext[:],
                    in_offset=bass.IndirectOffsetOnAxis(ap=dt_t[:, s:s + 1], axis=0))
                dl = sb.tile([P, CS], f32, tag="dl")
                nc.sync.dma_start(out=dl[:], in_=edloc[:, c0:c0 + CS])
                es = sb.tile([P, CS], i32, tag="es")
                nc.sync.dma_start(out=es[:], in_=esrc[:, c0:c0 + CS])

                G = sb.tile([P, CS * W1C], bf16, tag="G")
                for j in range(CS):
                    nc.gpsimd.indirect_dma_start(
                        out=G[:, j * W1C:(j + 1) * W1C], out_offset=None,
                        in_=h1ext[:],
                        in_offset=bass.IndirectOffsetOnAxis(ap=es[:, j:j + 1], axis=0))

                # one-hot for all chunks: M[p, j*128+d] = (dl[p,j] == d)
                M = sb.tile([P, CS * P], bf16, tag="M")
                nc.vector.tensor_tensor(
                    out=_ap(M, [M[:].ap[0], [P, CS], [1, P]]),
                    in0=_ap(dl, [dl[:].ap[0], [1, CS], [0, P]]),
                    in1=_ap(iota_f, [iota_f[:].ap[0], [0, CS], [1, P]]),
                    op=ALU.is_equal)

                # per-edge sdst: SD[:, 4j:4j+4] = (M_j)^T.T-free expand
                SD = pst.tile([P, CS * 4], f32, tag="SD")
                for j in range(CS):
                    pT = pst.tile([P, P], bf16, tag="pT")
                    nc.tensor.transpose(out=pT[:], in_=M[:, j * P:(j + 1) * P],
                                        identity=identb[:])
                    mt = sb.tile([P, P], bf16, tag="mt")
                    nc.vector.tensor_copy(out=mt[:], in_=pT[:])
                    nc.tensor.matmul(out=SD[:, j * 4:(j + 1) * 4], lhsT=mt[:],
                                     rhs=sdt[:, 132:136], start=True, stop=True)

                # scores -> ex, written back into G's cols 128:132 per block
                SDb = sb.tile([P, CS * 4], bf16, tag="SDb")
                nc.vector.tensor_copy(out=SDb[:], in_=SD[:])
                S = sb.tile([P, CS * 4], bf16, tag="S")
                nc.vector.tensor_tensor(
                    out=S[:],
                    in0=_ap(G, [G[:].ap[0], [W1C, CS], [1, 4]], extra_offset=128),
                    in1=SDb[:], op=ALU.add)
                Sm = sb.tile([P, CS * 4], bf16, tag="Sm")
                nc.vector.tensor_scalar(out=Sm[:], in0=S[:], scalar1=NEG_SLOPE,
                                        scalar2=None, op0=ALU.mult)
                nc.vector.tensor_tensor(out=S[:], in0=S[:], in1=Sm[:], op=ALU.max)
                EX = sb.tile([P, CS * 4], bf16, tag="EX")
                nc.scalar.activation(EX[:], S[:], AF.Exp)
                nc.vector.tensor_copy(
                    out=_ap(G, [G[:].ap[0], [W1C, CS], [1, 4]], extra_offset=128),
                    in_=EX[:])
                # scale features by per-(edge, head) ex
                nc.vector.tensor_tensor(
                    out=_ap(G, [G[:].ap[0], [W1C, CS], [32, 4], [1, 32]]),
                    in0=_ap(G, [G[:].ap[0], [W1C, CS], [32, 4], [1, 32]]),
                    in1=_ap(G, [G[:].ap[0], [W1C, CS], [1, 4], [0, 32]],
                            extra_offset=128),
                    op=ALU.mult)

                agg = psagg.tile([P, 132], f32, tag="agg")
                for j in range(CS):
                    nc.tensor.matmul(out=agg[:], lhsT=M[:, j * P:(j + 1) * P],
                                     rhs=G[:, j * W1C:j * W1C + 132],
                                     start=(j == 0), stop=(j == CS - 1))

                # epilogue: divide, elu, h2 = h @ W2cat, store shard row block
                if dbg and s == 0:
                    sdcp = sb.tile([P, CS * 4], f32, tag="sdcp")
                    nc.vector.tensor_copy(out=sdcp[:], in_=SD[:])
                    nc.sync.dma_start(out=sd_d[:], in_=sdcp[:])
                    nc.sync.dma_start(out=s_d[:], in_=S[:])
                    nc.sync.dma_start(out=ex_d[:], in_=EX[:])
                    nc.sync.dma_start(out=m_d[:], in_=M[:])
                    nc.sync.dma_start(out=g_d[:], in_=G[:])
                    nc.sync.dma_start(out=sdt_d[:], in_=sdt[:])
                    agcp = sb.tile([P, 132], f32, tag="agcp")
                    nc.vector.tensor_copy(out=agcp[:], in_=agg[:])
                    nc.sync.dma_start(out=agg_d[:], in_=agcp[:])
                den = sb.tile([P, 4], f32, tag="den")
                nc.vector.tensor_scalar(out=den[:], in0=agg[:, 128:132],
                                        scalar1=1e-30, scalar2=None, op0=ALU.max)
                rden = sb.tile([P, 4], f32, tag="rden")
                nc.vector.reciprocal(out=rden[:], in_=den[:])
                h_t = sb.tile([P, P], f32, tag="h_t")
                nc.vector.tensor_tensor(
                    out=_ap(h_t, [h_t[:].ap[0], [32, 4], [1, 32]]),
                    in0=_ap(agg, [agg[:].ap[0], [32, 4], [1, 32]]),
                    in1=_ap(rden, [rden[:].ap[0], [1, 4], [0, 32]]),
                    op=ALU.mult)
                # elu(x) = max(x,0) + exp(min(x,0)) - 1
                neg = sb.tile([P, P], f32, tag="neg")
                nc.vector.tensor_scalar(out=neg[:], in0=h_t[:], scalar1=0.0,
                                        scalar2=None, op0=ALU.min)
                eneg = sb.tile([P, P], f32, tag="eneg")
                nc.scalar.activation(eneg[:], neg[:], AF.Exp)
                nc.vector.tensor_scalar(out=h_t[:], in0=h_t[:], scalar1=0.0,
                                        scalar2=None, op0=ALU.max)
                nc.vector.tensor_tensor(out=h_t[:], in0=h_t[:], in1=eneg[:],
                                        op=ALU.add)
                nc.vector.tensor_scalar(out=h_t[:], in0=h_t[:], scalar1=-1.0,
                                        scalar2=None, op0=ALU.add)
                hT = pst.tile([P, P], f32, tag="pT")
                nc.tensor.transpose(out=hT[:], in_=h_t[:], identity=ident[:])
                hTs = sb.tile([P, P], f32, tag="hTs")
                nc.vector.tensor_copy(out=hTs[:], in_=hT[:])
                h2p = ps.tile([P, W2C], f32, tag="p0")
                nc.tensor.matmul(out=h2p[:], lhsT=hTs[:], rhs=w2_t[:],
                                 start=True, stop=True)
                h2s = sb.tile([P, W2C], bf16, tag="h2s")
                nc.scalar.copy(out=h2s[:], in_=h2p[:])
                nc.sync.dma_start(out=h2sh[s * P:(s + 1) * P, :], in_=h2s[:])

            if dbg:
                for bb in range(8):
                    dtt = sb.tile([P, W1C], bf16, tag="dbg1")
                    nc.sync.dma_start(out=dtt[:], in_=h1ext[bb * P:(bb + 1) * P, :])
                    nc.sync.dma_start(out=h1dbg[bb * P:(bb + 1) * P, :], in_=dtt[:])
                for bb in range(NS):
                    dt2 = sb.tile([P, W2C], bf16, tag="dbg2")
                    nc.sync.dma_start(out=dt2[:], in_=h2sh[bb * P:(bb + 1) * P, :])
                    nc.sync.dma_start(out=h2dbg[bb * P:(bb + 1) * P, :], in_=dt2[:])

            # ---------- AllGather shard tables
            nc.gpsimd.collective_compute(
                "AllGather", mybir.AluOpType.bypass,
                ins=[h2sh.opt()], outs=[h2full.opt()],
                replica_groups=[list(range(NC))])

            # ---------- layer 2 edge pass (same chunk structure)
            for s in range(ns_run):
                c0 = s * CS
                sdt2 = sb.tile([P, W2C], bf16, tag="sdt2")
                nc.gpsimd.indirect_dma_start(
                    out=sdt2[:], out_offset=None, in_=h2full[:],
                    in_offset=bass.IndirectOffsetOnAxis(ap=dt_t[:, s:s + 1], axis=0))
                dl = sb.tile([P, CS], f32, tag="dl")
                nc.sync.dma_start(out=dl[:], in_=edloc[:, c0:c0 + CS])
                es = sb.tile([P, CS], i32, tag="es")
                nc.sync.dma_start(out=es[:], in_=esrc[:, c0:c0 + CS])

                G2 = sb.tile([P, CS * W2C], bf16, tag="G2")
                for j in range(CS):
                    nc.gpsimd.indirect_dma_start(
                        out=G2[:, j * W2C:(j + 1) * W2C], out_offset=None,
                        in_=h2full[:],
                        in_offset=bass.IndirectOffsetOnAxis(ap=es[:, j:j + 1], axis=0))

                M = sb.tile([P, CS * P], bf16, tag="M")
                nc.vector.tensor_tensor(
                    out=_ap(M, [M[:].ap[0], [P, CS], [1, P]]),
                    in0=_ap(dl, [dl[:].ap[0], [1, CS], [0, P]]),
                    in1=_ap(iota_f, [iota_f[:].ap[0], [0, CS], [1, P]]),
                    op=ALU.is_equal)

                SD = pst.tile([P, CS], f32, tag="SD")
                for j in range(CS):
                    pT = pst.tile([P, P], bf16, tag="pT")
                    nc.tensor.transpose(out=pT[:], in_=M[:, j * P:(j + 1) * P],
                                        identity=identb[:])
                    mt = sb.tile([P, P], bf16, tag="mt")
                    nc.vector.tensor_copy(out=mt[:], in_=pT[:])
                    nc.tensor.matmul(out=SD[:, j:j + 1], lhsT=mt[:],
                                     rhs=sdt2[:, 33:34], start=True, stop=True)

                SDb2 = sb.tile([P, CS], bf16, tag="SDb")
                nc.vector.tensor_copy(out=SDb2[:], in_=SD[:])
                S = sb.tile([P, CS], bf16, tag="S2")
                nc.vector.tensor_tensor(
                    out=S[:],
                    in0=_ap(G2, [G2[:].ap[0], [W2C, CS], [1, 1]], extra_offset=32),
                    in1=SDb2[:], op=ALU.add)
                Sm2 = sb.tile([P, CS], bf16, tag="Sm")
                nc.vector.tensor_scalar(out=Sm2[:], in0=S[:], scalar1=NEG_SLOPE,
                                        scalar2=None, op0=ALU.mult)
                nc.vector.tensor_tensor(out=S[:], in0=S[:], in1=Sm2[:], op=ALU.max)
                EX2 = sb.tile([P, CS], bf16, tag="EX2")
                nc.scalar.activation(EX2[:], S[:], AF.Exp)
                nc.vector.tensor_copy(
                    out=_ap(G2, [G2[:].ap[0], [W2C, CS], [1, 1]], extra_offset=32),
                    in_=EX2[:])
                nc.vector.tensor_tensor(
                    out=_ap(G2, [G2[:].ap[0], [W2C, CS], [1, 32]]),
                    in0=_ap(G2, [G2[:].ap[0], [W2C, CS], [1, 32]]),
                    in1=_ap(G2, [G2[:].ap[0], [W2C, CS], [0, 32]],
                            extra_offset=32),
                    op=ALU.mult)

                agg2 = psagg.tile([P, 33], f32, tag="agg")
                for j in range(CS):
                    nc.tensor.matmul(out=agg2[:], lhsT=M[:, j * P:(j + 1) * P],
                                     rhs=G2[:, j * W2C:j * W2C + 33],
                                     start=(j == 0), stop=(j == CS - 1))

                den2 = sb.tile([P, 1], f32, tag="den2")
                nc.vector.tensor_scalar(out=den2[:], in0=agg2[:, 32:33],
                                        scalar1=1e-30, scalar2=None, op0=ALU.max)
                r2 = sb.tile([P, 1], f32, tag="r2")
                nc.vector.reciprocal(out=r2[:], in_=den2[:])
                o_t = sb.tile([P, OUT_DIM], f32, tag="o_t")
                nc.vector.tensor_scalar(out=o_t[:], in0=agg2[:, 0:32],
                                        scalar1=r2[:, 0:1], scalar2=None,
                                        op0=ALU.mult)
                nc.sync.dma_start(out=out2[s * P:(s + 1) * P, :], in_=o_t[:])

    nc.compile()
    return nc


def _install_ntff_shim():
    import contextlib
    import ctypes
    import types

    mod = types.ModuleType("antenv.axon_hooks")

    def _hook_factory(so_path="/opt/axon/libaxon_pjrt.so"):
        try:
            lib = ctypes.CDLL(so_path)
        except OSError:
            return None
        if not hasattr(lib, "axon_start_nrt_profile"):
            return None
        lib.axon_start_nrt_profile.argtypes = [
            ctypes.POINTER(ctypes.c_int64), ctypes.c_size_t]
        lib.axon_start_nrt_profile.restype = ctypes.c_int64
        lib.axon_stop_nrt_profile.argtypes = [ctypes.c_char_p]
        lib.axon_stop_nrt_profile.restype = ctypes.c_int64

        @contextlib.contextmanager
        def _hook(output_dir, device_ids):
            import jax
            jax.devices()
            if device_ids:
                ids = (ctypes.c_int64 * len(device_ids))(*device_ids)
                rc = lib.axon_start_nrt_profile(ids, len(device_ids))
            else:
                rc = lib.axon_start_nrt_profile(None, 0)
            if rc != 0:
                raise RuntimeError(f"axon_start_nrt_profile rc={rc}")
            try:
                yield
            finally:
                n = lib.axon_stop_nrt_profile(str(output_dir).encode())
                if n < 0:
                    raise RuntimeError(f"axon_stop_nrt_profile rc={n}")

        return _hook

    mod.get_axon_ntff_profile_hook = _hook_factory
    mod.set_axon_ntff_profile_hook = lambda h: None
    sys.modules["antenv.axon_hooks"] = mod
    from concourse import bass_utils as bu
    bu.upload_artifacts = lambda tmpdir: tmpdir


def _prep_inputs(x, edge_index, W1, a_src1, a_dst1, W2, a_src2, a_dst2):
    import ml_dtypes

    x = np.asarray(x, np.float32)
    ei = np.asarray(edge_index)
    src = np.concatenate([ei[0], np.arange(N, dtype=np.int64)]).astype(np.int64)
    dst = np.concatenate([ei[1], np.arange(N, dtype=np.int64)]).astype(np.int64)
    Etot = src.shape[0]

    tile_of = (dst >> 7).astype(np.int64)
    counts = np.bincount(tile_of, minlength=NT)
    if counts.max() > CS * P:
        raise ValueError(f"dst tile overflow: {counts.max()} > {CS * P}")
    order = np.argsort(tile_of, kind="stable")
    starts = np.zeros(NT, np.int64)
    np.cumsum(counts[:-1], out=starts[1:])
    tile_sorted = tile_of[order]
    pos = np.arange(Etot, dtype=np.int64) - starts[tile_sorted]

    src_pad = np.zeros((NT, CS * P), np.int32)
    dloc_pad = np.full((NT, CS * P), -1.0, np.float32)
    src_pad[tile_sorted, pos] = src[order].astype(np.int32)
    dloc_pad[tile_sorted, pos] = (dst[order] & 127).astype(np.float32)

    # weights
    W1 = np.asarray(W1, np.float32)                       # [128, 128]
    a_src1 = np.asarray(a_src1, np.float32)               # [4, 32]
    a_dst1 = np.asarray(a_dst1, np.float32)
    W1h = W1.reshape(IN_DIM, HEADS, HID)
    W1s = np.einsum("khc,hc->kh", W1h, a_src1)            # [128, 4]
    W1d = np.einsum("khc,hc->kh", W1h, a_dst1)
    W1cat = np.concatenate([W1, W1s, W1d], axis=1)        # [128, 136]

    W2 = np.asarray(W2, np.float32)                       # [128, 32]
    w2s = W2 @ np.asarray(a_src2, np.float32)[0]          # [128]
    w2d = W2 @ np.asarray(a_dst2, np.float32)[0]
    W2cat = np.concatenate(
        [W2, w2s[:, None], w2d[:, None], np.zeros((IN_DIM, 2), np.float32)],
        axis=1)                                           # [128, 36]

    xT = np.zeros((P, NPAD), np.float32)
    xT[:, :N] = x.T
    xT = xT.astype(ml_dtypes.bfloat16)
    W1cat_b = W1cat.astype(ml_dtypes.bfloat16)

    in_maps = []
    for c in range(NC):
        tiles = slice(c * NS, (c + 1) * NS)
        esrc_c = np.ascontiguousarray(
            src_pad[tiles].reshape(NS, CS, P).transpose(2, 0, 1).reshape(P, TC))
        edloc_c = np.ascontiguousarray(
            dloc_pad[tiles].reshape(NS, CS, P).transpose(2, 0, 1).reshape(P, TC))
        dtids_c = (c * SHARD + np.arange(NS)[None, :] * P
                   + np.arange(P)[:, None]).astype(np.int32)
        in_maps.append({
            "xT": xT, "W1cat": W1cat_b, "W2cat": W2cat,
            "esrc": esrc_c, "edloc": edloc_c, "dtids": dtids_c,
        })
    return in_maps


def kernel(**inputs):
    global _RUNNER
    from concourse.bass_utils import run_bass_kernel_spmd

    trace = os.environ.get("GAT_TRACE") == "1"
    if trace:
        _install_ntff_shim()

    if _RUNNER is None:
        if os.environ.get("GAT_SMOKE") == "1":
            _RUNNER = _build_program(ns_run=2, p0_groups=2)
        else:
            _RUNNER = _build_program()
    nc = _RUNNER

    in_maps = _prep_inputs(
        inputs["x"], inputs["edge_index"], inputs["W1"], inputs["a_src1"],
        inputs["a_dst1"], inputs["W2"], inputs["a_src2"], inputs["a_dst2"])

    kw = {}
    if trace:
        import tempfile
        kw = dict(trace=True, tmpdir=tempfile.mkdtemp())
    res = run_bass_kernel_spmd(nc, in_maps, list(range(NC)), **kw)
    if trace and res.exec_time_ns is not None:
        print(f"HW exec time: {res.exec_time_ns} ns")
        kernel.last_exec_time_ns = res.exec_time_ns

    full = np.concatenate([res.results[c]["out2"] for c in range(NC)], axis=0)
    out = full[:N] + np.asarray(inputs["b2"], np.float32)[None, :]
    return out.astype(np.float32)



# revision 10
# speedup vs baseline: 1.2475x; 1.2475x over previous
"""Two-layer GAT on 8 Trainium2 NeuronCores (Bass/Tile, SPMD) — v2.

Sharding: dst nodes split into 784 tiles of 128; core c owns 98 tiles.
Edges (WITHOUT the appended self-loops) are grouped by (dst tile, src
quadrant) and padded to CQ=640 slots per quadrant (empirical max 596),
giving 20 chunks of 128 edges per tile.  Source rows are fetched with
one dma_gather per (2-tile batch, quadrant) from a [NPAD, 256] bf16
table whose rows are [h1(128) | ssrc1(4) | sdst1(4) | pad]; int16
gather indices are quadrant-local (< 25088).  Per-edge dst scores come
from PE transposes of the one-hot M (as in v1).  The appended
self-loops are applied on-chip per dst node from persistent SBUF
copies of the core's own h1/h2 rows.  Segment max is dropped (scores
are O(1); softmax is shift-invariant).  Between layers the per-shard
h2 tables are AllGathered and restrided into a [NPAD, 128] bf16 table
for layer-2 gathers.
"""
import os
import sys

sys.path.insert(0, "/opt/trn_rl_repo")

import numpy as np

N = 100000
IN_DIM = 128
HID = 32
HEADS = 4
OUT_DIM = 32
NEG_SLOPE = 0.2

NC = 8
P = 128
NPAD = 100352          # 784 tiles of 128
SHARD = NPAD // NC     # 12544
NS = SHARD // P        # 98 dst tiles per core
NT = NPAD // P         # 784
Q = 4                  # src quadrants (int16 gather indices)
QS = NPAD // Q         # 25088 rows per quadrant subtable
CQ = 640               # slots per (tile, quadrant); empirical max 596
CPQ = CQ // P          # 5 chunks per quadrant
CS = Q * CPQ           # 20 chunks of 128 edges per tile
SLOTS = CS * P         # 2560 edge slots per tile
B = 2                  # tiles per dma_gather batch
NB = NS // B           # 49 batches per core
NIB = B * CQ           # 1280 gather indices per instruction
W1C = 136              # h1(128) | ssrc1(4) | sdst1(4)
TABC = 256             # h1 table row (bf16) = 512 bytes
W2C = 36               # h2(32) | ssrc2(1) | sdst2(1) | pad(2)
TAB2C = 128            # h2 table row (bf16) = 256 bytes

_RUNNER = None


def _ap(t, ap_dims, extra_offset=0):
    import concourse.bass as bass
    base = t[:]
    return bass.AP(base.tensor, base.offset + extra_offset, ap_dims)


def _sap(apobj, ap_dims, extra_offset=0):
    """AP from an existing AP (e.g. a tile slice), with new free dims."""
    import concourse.bass as bass
    return bass.AP(apobj.tensor, apobj.offset + extra_offset, ap_dims)


def _build_program(ns_run=NS, p0_groups=None):
    stage = os.environ.get("GAT_STAGE", "all")
    from concourse import bass, mybir, bacc
    import concourse.tile as tile
    from concourse.masks import make_identity

    f32 = mybir.dt.float32
    bf16 = mybir.dt.bfloat16
    i16 = mybir.dt.int16
    i32 = mybir.dt.int32
    AF = mybir.ActivationFunctionType
    ALU = mybir.AluOpType

    nb_run = ns_run // B

    nc = bacc.Bacc("TRN2", target_bir_lowering=False, debug=False, num_devices=NC)

    xT = nc.dram_tensor("xT", [P, NPAD], bf16, kind="ExternalInput")
    W1cat = nc.dram_tensor("W1cat", [P, W1C], bf16, kind="ExternalInput")
    W2cat = nc.dram_tensor("W2cat", [P, W2C], bf16, kind="ExternalInput")
    csum2 = nc.dram_tensor("csum2", [P, W2C], f32, kind="ExternalInput")
    esg = nc.dram_tensor("esg", [P, Q * NB * (NIB // 16)], i16, kind="ExternalInput")
    edl = nc.dram_tensor("edl", [P, NS * CS], f32, kind="ExternalInput")
    dtids = nc.dram_tensor("dtids", [P, NS], i32, kind="ExternalInput")
    out2 = nc.dram_tensor("out2", [SHARD, OUT_DIM], f32, kind="ExternalOutput")

    h1tab = nc.dram_tensor("h1tab", [NPAD, TABC], bf16)
    h2sh = nc.dram_tensor("h2sh", [SHARD, W2C], bf16)
    h2f36 = nc.dram_tensor("h2f36", [NPAD, W2C], bf16)
    h2tab = nc.dram_tensor("h2tab", [NPAD, TAB2C], bf16)

    ESGQ = NB * (NIB // 16)   # esg cols per quadrant

    with tile.TileContext(nc) as tc:
        with (
            tc.tile_pool(name="consts", bufs=1) as consts,
            tc.tile_pool(name="persist", bufs=1) as persist,
            tc.tile_pool(name="sb", bufs=4) as sb,
            tc.tile_pool(name="mr", bufs=2) as mr,
            tc.tile_pool(name="gx", bufs=3) as gx,
            tc.tile_pool(name="gg", bufs=2) as gg,
            tc.tile_pool(name="ps", bufs=2, space="PSUM") as ps,
            tc.tile_pool(name="pst", bufs=2, space="PSUM") as pst,
            tc.tile_pool(name="psagg", bufs=2, space="PSUM") as psagg,
        ):
            ident = consts.tile([P, P], f32)
            make_identity(nc, ident[:])
            identb = consts.tile([P, P], bf16)
            nc.vector.tensor_copy(out=identb[:], in_=ident[:])
            iota_i = consts.tile([P, P], i32)
            nc.gpsimd.iota(iota_i[:], pattern=[[1, P]], base=0, channel_multiplier=0)
            iota_f = consts.tile([P, P], f32)
            nc.vector.tensor_copy(out=iota_f[:], in_=iota_i[:])
            w1_t = consts.tile([P, W1C], bf16)
            nc.sync.dma_start(out=w1_t[:], in_=W1cat[:])
            w2_t = consts.tile([P, W2C], bf16)
            nc.sync.dma_start(out=w2_t[:], in_=W2cat[:])
            cs2_t = consts.tile([P, W2C], f32)
            nc.sync.dma_start(out=cs2_t[:], in_=csum2[:])

            esg_t = persist.tile([P, Q * ESGQ], i16)
            nc.sync.dma_start(out=esg_t[:], in_=esg[:])
            edl_t = persist.tile([P, NS * CS], f32)
            nc.sync.dma_start(out=edl_t[:], in_=edl[:])
            dt_t = persist.tile([P, NS], i32)
            nc.sync.dma_start(out=dt_t[:], in_=dtids[:])
            own1 = persist.tile([P, NS * W1C], bf16)
            own2 = persist.tile([P, NS * W2C], bf16)

            # ---------- phase 0: h1tab rows = [x@W1 | x@W1s | x@W1d]
            GRP = 8
            _ng = NT // GRP if p0_groups is None else p0_groups
            for g in range(_ng):
                xg = gx.tile([P, P * GRP], bf16, tag="xg")
                nc.sync.dma_start(out=xg[:], in_=xT[:, g * P * GRP:(g + 1) * P * GRP])
                s0g = gx.tile([P, GRP * W1C], bf16, tag="s0g")
                for t in range(GRP):
                    p0 = ps.tile([P, W1C], f32, tag="p0")
                    nc.tensor.matmul(out=p0[:], lhsT=xg[:, t * P:(t + 1) * P],
                                     rhs=w1_t[:], start=True, stop=True)
                    nc.scalar.copy(out=s0g[:, t * W1C:(t + 1) * W1C], in_=p0[:])
                nc.sync.dma_start(
                    out=_ap(h1tab, [[TABC, P], [TABC * P, GRP], [1, W1C]],
                            extra_offset=g * GRP * P * TABC),
                    in_=s0g[:])

            # ---------- layer 1 edge pass
            for b in range(nb_run if stage not in ("p0",) else 0):
                # own-shard rows for this batch's tiles (h1|ssrc1|sdst1)
                for tt in range(B):
                    t = b * B + tt
                    nc.gpsimd.indirect_dma_start(
                        out=own1[:, t * W1C:(t + 1) * W1C], out_offset=None,
                        in_=h1tab[:],
                        in_offset=bass.IndirectOffsetOnAxis(
                            ap=dt_t[:, t:t + 1], axis=0))
                if stage == "own":
                    continue
                G = gg.tile([P, Q * B * CPQ, TABC], bf16, tag="G")
                for q in range(Q):
                    nc.gpsimd.dma_gather(
                        G[:, q * B * CPQ:(q + 1) * B * CPQ, :],
                        h1tab[q * QS:(q + 1) * QS, :],
                        esg_t[:, q * ESGQ + b * (NIB // 16):
                              q * ESGQ + (b + 1) * (NIB // 16)],
                        NIB, NIB, TABC, single_packet=False)
                if stage == "l1g":
                    continue
                for tt in range(B):
                    t = b * B + tt
                    dl = edl_t[:, t * CS:(t + 1) * CS]
                    M = mr.tile([P, CS * P], bf16, tag="M")
                    nc.vector.tensor_tensor(
                        out=_ap(M, [M[:].ap[0], [P, CS], [1, P]]),
                        in0=_sap(dl, [dl.ap[0], [1, CS], [0, P]]),
                        in1=_ap(iota_f, [iota_f[:].ap[0], [0, CS], [1, P]]),
                        op=ALU.is_equal)

                    # per-edge sdst via PE transpose of M chunks
                    sdt = own1[:, t * W1C + 132:t * W1C + 136]
                    SD = pst.tile([P, CS * 4], f32, tag="SD")
                    for j in range(CS):
                        pT = pst.tile([P, P], bf16, tag="pT")
                        nc.tensor.transpose(out=pT[:], in_=M[:, j * P:(j + 1) * P],
                                            identity=identb[:])
                        mt = sb.tile([P, P], bf16, tag="mt")
                        if j % 2 == 0:
                            nc.vector.tensor_copy(out=mt[:], in_=pT[:])
                        else:
                            nc.scalar.copy(out=mt[:], in_=pT[:])
                        nc.tensor.matmul(out=SD[:, j * 4:(j + 1) * 4], lhsT=mt[:],
                                         rhs=sdt, start=True, stop=True)
                    SDb = sb.tile([P, CS * 4], bf16, tag="SDb")
                    nc.scalar.copy(out=SDb[:], in_=SD[:])

                    # S = ssrc(G) + SD ; lrelu ; exp
                    goff = tt * CPQ * TABC
                    S = sb.tile([P, CS * 4], bf16, tag="S")
                    nc.vector.tensor_tensor(
                        out=_ap(S, [S[:].ap[0], [CPQ * 4, Q], [4, CPQ], [1, 4]]),
                        in0=_ap(G, [G[:].ap[0], [B * CPQ * TABC, Q], [TABC, CPQ],
                                    [1, 4]], extra_offset=goff + 128),
                        in1=_ap(SDb, [SDb[:].ap[0], [CPQ * 4, Q], [4, CPQ], [1, 4]]),
                        op=ALU.add)
                    Sm = sb.tile([P, CS * 4], bf16, tag="Sm")
                    nc.vector.tensor_scalar(out=Sm[:], in0=S[:], scalar1=NEG_SLOPE,
                                            scalar2=None, op0=ALU.mult)
                    nc.vector.tensor_tensor(out=S[:], in0=S[:], in1=Sm[:], op=ALU.max)
                    EX = sb.tile([P, CS * 4], bf16, tag="EX")
                    nc.scalar.activation(EX[:], S[:], AF.Exp)

                    # R = [ex-scaled feats | ex] per chunk (132 cols)
                    R = mr.tile([P, CS * 132], bf16, tag="R")
                    for q in range(Q):
                        nc.vector.tensor_tensor(
                            out=_ap(R, [R[:].ap[0], [132, CPQ], [32, 4], [1, 32]],
                                    extra_offset=q * CPQ * 132),
                            in0=_ap(G, [G[:].ap[0], [TABC, CPQ], [32, 4], [1, 32]],
                                    extra_offset=(q * B * CPQ + tt * CPQ) * TABC),
                            in1=_ap(EX, [EX[:].ap[0], [4, CPQ], [1, 4], [0, 32]],
                                    extra_offset=q * CPQ * 4),
                            op=ALU.mult)
                    nc.scalar.copy(
                        out=_ap(R, [R[:].ap[0], [132, CS], [1, 4]], extra_offset=128),
                        in_=_ap(EX, [EX[:].ap[0], [4, CS], [1, 4]]))

                    agg = psagg.tile([P, 132], f32, tag="agg")
                    for j in range(CS):
                        nc.tensor.matmul(out=agg[:], lhsT=M[:, j * P:(j + 1) * P],
                                         rhs=R[:, j * 132:j * 132 + 132],
                                         start=(j == 0), stop=(j == CS - 1))

                    # self-loop term from own rows
                    o1 = t * W1C
                    Ss = sb.tile([P, 4], bf16, tag="Ss")
                    nc.vector.tensor_tensor(
                        out=Ss[:], in0=own1[:, o1 + 128:o1 + 132],
                        in1=own1[:, o1 + 132:o1 + 136], op=ALU.add)
                    Ssm = sb.tile([P, 4], bf16, tag="Ssm")
                    nc.vector.tensor_scalar(out=Ssm[:], in0=Ss[:], scalar1=NEG_SLOPE,
                                            scalar2=None, op0=ALU.mult)
                    nc.vector.tensor_tensor(out=Ss[:], in0=Ss[:], in1=Ssm[:],
                                            op=ALU.max)
                    EXs = sb.tile([P, 4], bf16, tag="EXs")
                    nc.scalar.activation(EXs[:], Ss[:], AF.Exp)
                    tmp = sb.tile([P, P], f32, tag="tmp")
                    nc.vector.tensor_tensor(
                        out=_ap(tmp, [tmp[:].ap[0], [32, 4], [1, 32]]),
                        in0=_ap(own1, [own1[:].ap[0], [32, 4], [1, 32]],
                                extra_offset=o1),
                        in1=_ap(EXs, [EXs[:].ap[0], [1, 4], [0, 32]]),
                        op=ALU.mult)

                    den = sb.tile([P, 4], f32, tag="den")
                    nc.vector.tensor_tensor(out=den[:], in0=agg[:, 128:132],
                                            in1=EXs[:], op=ALU.add)
                    nc.vector.tensor_scalar(out=den[:], in0=den[:],
                                            scalar1=1e-30, scalar2=None, op0=ALU.max)
                    rden = sb.tile([P, 4], f32, tag="rden")
                    nc.vector.reciprocal(out=rden[:], in_=den[:])

                    h_t = sb.tile([P, P], f32, tag="h_t")
                    nc.vector.tensor_tensor(out=h_t[:], in0=agg[:, 0:128],
                                            in1=tmp[:], op=ALU.add)
                    nc.vector.tensor_tensor(
                        out=_ap(h_t, [h_t[:].ap[0], [32, 4], [1, 32]]),
                        in0=_ap(h_t, [h_t[:].ap[0], [32, 4], [1, 32]]),
                        in1=_ap(rden, [rden[:].ap[0], [1, 4], [0, 32]]),
                        op=ALU.mult)
                    # elu(x)+1 = max(x,0) + exp(min(x,0)); -1 folded into csum2
                    neg = sb.tile([P, P], f32, tag="neg")
                    nc.vector.tensor_scalar(out=neg[:], in0=h_t[:], scalar1=0.0,
                                            scalar2=None, op0=ALU.min)
                    eneg = sb.tile([P, P], f32, tag="eneg")
                    nc.scalar.activation(eneg[:], neg[:], AF.Exp)
                    nc.vector.tensor_scalar(out=h_t[:], in0=h_t[:], scalar1=0.0,
                                            scalar2=None, op0=ALU.max)
                    y = sb.tile([P, P], bf16, tag="y")
                    nc.vector.tensor_tensor(out=y[:], in0=h_t[:], in1=eneg[:],
                                            op=ALU.add)
                    yT = pst.tile([P, P], bf16, tag="pT")
                    nc.tensor.transpose(out=yT[:], in_=y[:], identity=identb[:])
                    yTs = sb.tile([P, P], bf16, tag="yTs")
                    nc.vector.tensor_copy(out=yTs[:], in_=yT[:])
                    h2p = ps.tile([P, W2C], f32, tag="p0")
                    nc.tensor.matmul(out=h2p[:], lhsT=yTs[:], rhs=w2_t[:],
                                     start=True, stop=True)
                    h2s = sb.tile([P, W2C], bf16, tag="h2s")
                    nc.vector.tensor_tensor(out=h2s[:], in0=h2p[:], in1=cs2_t[:],
                                            op=ALU.subtract)
                    nc.sync.dma_start(out=h2sh[t * P:(t + 1) * P, :], in_=h2s[:])
                    nc.scalar.copy(out=own2[:, t * W2C:(t + 1) * W2C], in_=h2s[:])

            # ---------- AllGather + restride into gather table
            if stage in ("ag", "all"):
                nc.gpsimd.collective_compute(
                    "AllGather", mybir.AluOpType.bypass,
                    ins=[h2sh[:]], outs=[h2f36[:]],
                    replica_groups=[list(range(NC))])
                for rr in range(NC):
                    nc.sync.dma_start(
                        out=_ap(h2tab, [[TAB2C, SHARD], [1, W2C]],
                                extra_offset=rr * SHARD * TAB2C),
                        in_=h2f36[rr * SHARD:(rr + 1) * SHARD, :])

            # ---------- layer 2 edge pass
            for b in range(nb_run if stage == "all" else 0):
                G2 = gg.tile([P, Q * B * CPQ, TAB2C], bf16, tag="G2")
                for q in range(Q):
                    nc.gpsimd.dma_gather(
                        G2[:, q * B * CPQ:(q + 1) * B * CPQ, :],
                        h2tab[q * QS:(q + 1) * QS, :],
                        esg_t[:, q * ESGQ + b * (NIB // 16):
                              q * ESGQ + (b + 1) * (NIB // 16)],
                        NIB, NIB, TAB2C, single_packet=False)
                for tt in range(B):
                    t = b * B + tt
                    dl = edl_t[:, t * CS:(t + 1) * CS]
                    M = mr.tile([P, CS * P], bf16, tag="M")
                    nc.vector.tensor_tensor(
                        out=_ap(M, [M[:].ap[0], [P, CS], [1, P]]),
                        in0=_sap(dl, [dl.ap[0], [1, CS], [0, P]]),
                        in1=_ap(iota_f, [iota_f[:].ap[0], [0, CS], [1, P]]),
                        op=ALU.is_equal)

                    sdt2 = own2[:, t * W2C + 33:t * W2C + 34]
                    SD = pst.tile([P, CS], f32, tag="SD")
                    for j in range(CS):
                        pT = pst.tile([P, P], bf16, tag="pT")
                        nc.tensor.transpose(out=pT[:], in_=M[:, j * P:(j + 1) * P],
                                            identity=identb[:])
                        mt = sb.tile([P, P], bf16, tag="mt")
                        if j % 2 == 0:
                            nc.vector.tensor_copy(out=mt[:], in_=pT[:])
                        else:
                            nc.scalar.copy(out=mt[:], in_=pT[:])
                        nc.tensor.matmul(out=SD[:, j:j + 1], lhsT=mt[:],
                                         rhs=sdt2, start=True, stop=True)
                    SDb = sb.tile([P, CS], bf16, tag="SDb2")
                    nc.scalar.copy(out=SDb[:], in_=SD[:])

                    goff = tt * CPQ * TAB2C
                    S = sb.tile([P, CS], bf16, tag="S2")
                    nc.vector.tensor_tensor(
                        out=_ap(S, [S[:].ap[0], [CPQ, Q], [1, CPQ]]),
                        in0=_ap(G2, [G2[:].ap[0], [B * CPQ * TAB2C, Q],
                                     [TAB2C, CPQ]], extra_offset=goff + 32),
                        in1=_ap(SDb, [SDb[:].ap[0], [CPQ, Q], [1, CPQ]]),
                        op=ALU.add)
                    Sm = sb.tile([P, CS], bf16, tag="Sm2")
                    nc.vector.tensor_scalar(out=Sm[:], in0=S[:], scalar1=NEG_SLOPE,
                                            scalar2=None, op0=ALU.mult)
                    nc.vector.tensor_tensor(out=S[:], in0=S[:], in1=Sm[:], op=ALU.max)
                    EX = sb.tile([P, CS], bf16, tag="EX2")
                    nc.scalar.activation(EX[:], S[:], AF.Exp)

                    R = mr.tile([P, CS * 33], bf16, tag="R2")
                    for q in range(Q):
                        nc.vector.tensor_tensor(
                            out=_ap(R, [R[:].ap[0], [33, CPQ], [1, 32]],
                                    extra_offset=q * CPQ * 33),
                            in0=_ap(G2, [G2[:].ap[0], [TAB2C, CPQ], [1, 32]],
                                    extra_offset=(q * B * CPQ + tt * CPQ) * TAB2C),
                            in1=_ap(EX, [EX[:].ap[0], [1, CPQ], [0, 32]],
                                    extra_offset=q * CPQ),
                            op=ALU.mult)
                    nc.scalar.copy(
                        out=_ap(R, [R[:].ap[0], [33, CS], [1, 1]], extra_offset=32),
                        in_=_ap(EX, [EX[:].ap[0], [1, CS], [1, 1]]))

                    agg = psagg.tile([P, 33], f32, tag="agg")
                    for j in range(CS):
                        nc.tensor.matmul(out=agg[:], lhsT=M[:, j * P:(j + 1) * P],
                                         rhs=R[:, j * 33:j * 33 + 33],
                                         start=(j == 0), stop=(j == CS - 1))

                    o2 = t * W2C
                    Ss = sb.tile([P, 1], bf16, tag="Ss2")
                    nc.vector.tensor_tensor(
                        out=Ss[:], in0=own2[:, o2 + 32:o2 + 33],
                        in1=own2[:, o2 + 33:o2 + 34], op=ALU.add)
                    Ssm = sb.tile([P, 1], bf16, tag="Ssm2")
                    nc.vector.tensor_scalar(out=Ssm[:], in0=Ss[:], scalar1=NEG_SLOPE,
                                            scalar2=None, op0=ALU.mult)
                    nc.vector.tensor_tensor(out=Ss[:], in0=Ss[:], in1=Ssm[:],
                                            op=ALU.max)
                    EXs = sb.tile([P, 1], f32, tag="EXs2")
                    nc.scalar.activation(EXs[:], Ss[:], AF.Exp)
                    tmp = sb.tile([P, OUT_DIM], f32, tag="tmp2")
                    nc.vector.tensor_scalar(
                        out=tmp[:], in0=own2[:, o2:o2 + 32],
                        scalar1=EXs[:, 0:1], scalar2=None, op0=ALU.mult)

                    den = sb.tile([P, 1], f32, tag="den2")
                    nc.vector.tensor_tensor(out=den[:], in0=agg[:, 32:33],
                                            in1=EXs[:], op=ALU.add)
                    nc.vector.tensor_scalar(out=den[:], in0=den[:], scalar1=1e-30,
                                            scalar2=None, op0=ALU.max)
                    r2 = sb.tile([P, 1], f32, tag="r2")
                    nc.vector.reciprocal(out=r2[:], in_=den[:])
                    o_t = sb.tile([P, OUT_DIM], f32, tag="o_t")
                    nc.vector.tensor_tensor(out=o_t[:], in0=agg[:, 0:32],
                                            in1=tmp[:], op=ALU.add)
                    nc.vector.tensor_scalar(out=o_t[:], in0=o_t[:],
                                            scalar1=r2[:, 0:1], scalar2=None,
                                            op0=ALU.mult)
                    nc.sync.dma_start(out=out2[t * P:(t + 1) * P, :], in_=o_t[:])

    nc.compile()
    return nc


def _install_ntff_shim():
    import contextlib
    import ctypes
    import types

    mod = types.ModuleType("antenv.axon_hooks")

    def _hook_factory(so_path="/opt/axon/libaxon_pjrt.so"):
        try:
            lib = ctypes.CDLL(so_path)
        except OSError:
            return None
        if not hasattr(lib, "axon_start_nrt_profile"):
            return None
        lib.axon_start_nrt_profile.argtypes = [
            ctypes.POINTER(ctypes.c_int64), ctypes.c_size_t]
        lib.axon_start_nrt_profile.restype = ctypes.c_int64
        lib.axon_stop_nrt_profile.argtypes = [ctypes.c_char_p]
        lib.axon_stop_nrt_profile.restype = ctypes.c_int64

        @contextlib.contextmanager
        def _hook(output_dir, device_ids):
            import jax
            jax.devices()
            if device_ids:
                ids = (ctypes.c_int64 * len(device_ids))(*device_ids)
                rc = lib.axon_start_nrt_profile(ids, len(device_ids))
            else:
                rc = lib.axon_start_nrt_profile(None, 0)
            if rc != 0:
                raise RuntimeError(f"axon_start_nrt_profile rc={rc}")
            try:
                yield
            finally:
                n = lib.axon_stop_nrt_profile(str(output_dir).encode())
                if n < 0:
                    raise RuntimeError(f"axon_stop_nrt_profile rc={n}")

        return _hook

    mod.get_axon_ntff_profile_hook = _hook_factory
    mod.set_axon_ntff_profile_hook = lambda h: None
    sys.modules["antenv.axon_hooks"] = mod
    from concourse import bass_utils as bu
    bu.upload_artifacts = lambda tmpdir: tmpdir


def _prep_inputs(x, edge_index, W1, a_src1, a_dst1, W2, a_src2, a_dst2):
    import ml_dtypes

    x = np.asarray(x, np.float32)
    ei = np.asarray(edge_index)
    src = ei[0].astype(np.int64)
    dst = ei[1].astype(np.int64)
    E = src.shape[0]

    tile_of = dst >> 7
    dl = (dst & 127).astype(np.float32)
    quad = src // QS
    srcq = (src - quad * QS).astype(np.int16)

    gkey = tile_of * Q + quad
    counts = np.bincount(gkey, minlength=NT * Q)
    if counts.max() > CQ:
        raise ValueError(f"tile-quadrant overflow: {counts.max()} > {CQ}")
    order = np.argsort(gkey, kind="stable")
    starts = np.zeros(NT * Q, np.int64)
    np.cumsum(counts[:-1], out=starts[1:])
    pos = np.arange(E, dtype=np.int64) - starts[gkey[order]]
    gk = gkey[order]
    slot = (gk // Q) * SLOTS + (gk % Q) * CQ + pos

    esrc_slots = np.zeros(NT * SLOTS, np.int16)
    edl_slots = np.full(NT * SLOTS, -1.0, np.float32)
    esrc_slots[slot] = srcq[order]
    edl_slots[slot] = dl[order]

    W1 = np.asarray(W1, np.float32)
    a_src1 = np.asarray(a_src1, np.float32)
    a_dst1 = np.asarray(a_dst1, np.float32)
    W1h = W1.reshape(IN_DIM, HEADS, HID)
    W1s = np.einsum("khc,hc->kh", W1h, a_src1)
    W1d = np.einsum("khc,hc->kh", W1h, a_dst1)
    W1cat = np.concatenate([W1, W1s, W1d], axis=1).astype(ml_dtypes.bfloat16)

    W2 = np.asarray(W2, np.float32)
    w2s = W2 @ np.asarray(a_src2, np.float32)[0]
    w2d = W2 @ np.asarray(a_dst2, np.float32)[0]
    W2cat = np.concatenate(
        [W2, w2s[:, None], w2d[:, None], np.zeros((IN_DIM, 2), np.float32)],
        axis=1)
    csum2 = np.tile(W2cat.sum(axis=0, keepdims=True), (P, 1)).astype(np.float32)
    W2cat_b = W2cat.astype(ml_dtypes.bfloat16)

    xT = np.zeros((P, NPAD), np.float32)
    xT[:, :N] = x.T
    xT = xT.astype(ml_dtypes.bfloat16)

    esrc_t = esrc_slots.reshape(NT, SLOTS)
    edl_t = edl_slots.reshape(NT, SLOTS)

    in_maps = []
    for c in range(NC):
        t0 = c * NS
        # gather stream for (q, b): i in [0, NIB): tile tt=i//CQ, rank i%CQ
        sl = esrc_t[t0:t0 + NS].reshape(NB, B, Q, CQ)
        sl = sl.transpose(2, 0, 1, 3).reshape(Q, NB, NIB)
        wrap = sl.reshape(Q, NB, NIB // 16, 16).transpose(0, 3, 1, 2).reshape(
            Q, 16, NB * (NIB // 16))
        esg_c = np.concatenate(
            [np.tile(wrap[q], (8, 1)) for q in range(Q)], axis=1).astype(np.int16)
        esg_c = np.ascontiguousarray(esg_c)

        ed = edl_t[t0:t0 + NS].reshape(NS, CS, P).transpose(2, 0, 1).reshape(
            P, NS * CS)
        edl_c = np.ascontiguousarray(ed)

        dtids_c = (c * SHARD + np.arange(NS)[None, :] * P
                   + np.arange(P)[:, None]).astype(np.int32)
        in_maps.append({
            "xT": xT, "W1cat": W1cat, "W2cat": W2cat_b, "csum2": csum2,
            "esg": esg_c, "edl": edl_c, "dtids": dtids_c,
        })
    return in_maps


def kernel(**inputs):
    global _RUNNER
    from concourse.bass_utils import run_bass_kernel_spmd

    trace = os.environ.get("GAT_TRACE") == "1"
    if trace:
        _install_ntff_shim()

    if _RUNNER is None:
        if os.environ.get("GAT_SMOKE") == "1":
            _RUNNER = _build_program(ns_run=2, p0_groups=2)
        else:
            _RUNNER = _build_program()
    nc = _RUNNER

    in_maps = _prep_inputs(
        inputs["x"], inputs["edge_index"], inputs["W1"], inputs["a_src1"],
        inputs["a_dst1"], inputs["W2"], inputs["a_src2"], inputs["a_dst2"])

    kw = {}
    if trace:
        import tempfile
        kw = dict(trace=True, tmpdir=tempfile.mkdtemp())
    res = run_bass_kernel_spmd(nc, in_maps, list(range(NC)), **kw)
    if trace and res.exec_time_ns is not None:
        print(f"HW exec time: {res.exec_time_ns} ns")
        kernel.last_exec_time_ns = res.exec_time_ns

    full = np.concatenate([res.results[c]["out2"] for c in range(NC)], axis=0)
    out = full[:N] + np.asarray(inputs["b2"], np.float32)[None, :]
    return out.astype(np.float32)
